# revision 8
# baseline (speedup 1.0000x reference)
"""Trainium2 Bass kernel for nn_CrossAttention2 (8 cores, data-parallel over batch).

Reference computation (per batch element b):
    q = Wq @ x_b + bq          # [512, 1024]   x_b = x[b].reshape(512, 32*32)
    k = Wk @ c_b + bk          # c_b = context[b]
    v = Wv @ c_b + bv
    per head h (8 heads x 64 dim):
        S_h = (Q_h^T @ K_h) / 8        # [1024q, 1024k]
        A_h = softmax(S_h, axis=k)
        out_h = V_h @ A_h              # contraction over the QUERY axis (faithful
                                       # to the original module's quirk)
    y_b = Wo @ concat(out_h) + bo

Sharding: one batch element per NeuronCore (BS == n_cores == 8), no collectives.

All inputs are packed host-side into ONE [128, BLOB_F] float32 blob per core so
the kernel issues a single input DMA (DMA instructions have high fixed latency
on this runtime) and a single output DMA.

Blob layout (free-dim offsets, per partition p):
    [    0,  4096)  xT      x[kc*128+p, hw]       -> 4 chunks * 1024
    [ 4096,  8192)  ctxT
    [ 8192, 10240)  wqT     Wq.T/8 [kc*128+p, i]  -> 4 chunks * 512
    [10240, 12288)  wkT
    [12288, 14336)  wvT
    [14336, 16384)  woT
    [16384, 16388)  bq/8    [p, m] (inner = m*128+p)
    [16388, 16392)  bk
    [16392, 16396)  bo
    [16396, 16908)  bv      row: partition 0 only
    [16908, 17036)  ones    row of 1.0: partition 0 only
BLOB_F = 17056 (padded to a multiple of 32)

Matmul dtype is float32r (1 row/cycle at N=512 on the PE; rel err ~1e-4).
Scores psum [128 q, 1024 k]; softmax skips max-subtraction (scores are O(+-6);
exp cannot overflow; softmax is shift-invariant). Normalization is folded into
V^T rows (16x fewer elements than scaling probabilities).
"""

import numpy as np
from contextlib import ExitStack

import concourse.bass as bass
from concourse import bacc
import concourse.tile as tile
from concourse import mybir
from concourse.bass_utils import run_bass_kernel_spmd

F32 = mybir.dt.float32
F32R = mybir.dt.float32r

BS, C, H, W = 8, 512, 32, 32
HW = H * W
N_HEADS, DIM_HEAD = 8, 64
INNER = N_HEADS * DIM_HEAD
N_CORES = 8

MM_DT = F32R

OFF_X = 0
OFF_CTX = 4096
OFF_WQ = 8192
OFF_WK = 10240
OFF_WV = 12288
OFF_WO = 14336
OFF_BQ = 16384
OFF_BK = 16388
OFF_BO = 16392
OFF_BV = 16396
OFF_ONES = 16908
BLOB_F = 17056


def _mm(nc, out, lhsT, rhs, start, stop):
    nc.tensor.matmul(out, lhsT, rhs, start=start, stop=stop)


def make_pools(ctx: ExitStack, tc: tile.TileContext):
    const = ctx.enter_context(tc.tile_pool(name="const", bufs=1))
    # PSUM: 8 banks of 2KB. sp: [128,1024] f32 = 2 banks x 3; av: 2 banks x 1.
    sp = ctx.enter_context(tc.tile_pool(name="sp", bufs=3, space="PSUM"))
    av = ctx.enter_context(tc.tile_pool(name="av", bufs=1, space="PSUM"))
    sm = ctx.enter_context(tc.tile_pool(name="sm", bufs=8))
    ppool = ctx.enter_context(tc.tile_pool(name="probs", bufs=3))
    vpool = ctx.enter_context(tc.tile_pool(name="vsc", bufs=4))
    return const, sp, av, sm, ppool, vpool


def _kernel_body(ctx: ExitStack, tc: tile.TileContext, io: dict, pools=None,
                 variant="full"):
    nc = tc.nc
    if pools is None:
        pools = make_pools(ctx, tc)
    const, sp, av, sm, ppool, vpool = pools

    blob = const.tile([128, BLOB_F], MM_DT, tag="blob")
    nc.sync.dma_start(out=blob[:], in_=io["blob"])

    def seg(off, ln):
        return blob[:, off:off + ln]

    Xs = seg(OFF_X, 4096).rearrange("p (kc f) -> p kc f", kc=4)
    Cs = seg(OFF_CTX, 4096).rearrange("p (kc f) -> p kc f", kc=4)
    WqT = seg(OFF_WQ, 2048).rearrange("p (kc f) -> p kc f", kc=4)
    WkT = seg(OFF_WK, 2048).rearrange("p (kc f) -> p kc f", kc=4)
    WvT = seg(OFF_WV, 2048).rearrange("p (kc f) -> p kc f", kc=4)
    WoT = seg(OFF_WO, 2048).rearrange("p (kc f) -> p kc f", kc=4)
    bq = seg(OFF_BQ, 4).bitcast(F32)
    bk = seg(OFF_BK, 4).bitcast(F32)
    bo = seg(OFF_BO, 4).bitcast(F32)
    bv = blob[0:1, OFF_BV:OFF_BV + 512]
    ones = blob[0:1, OFF_ONES:OFF_ONES + 128]

    Q = const.tile([128, 4, 1024], MM_DT, tag="Q")
    K = const.tile([128, 4, 1024], MM_DT, tag="K")
    Vt = const.tile([128, 8, 512], MM_DT, tag="Vt")
    O = const.tile([128, 4, 1024], MM_DT, tag="O")
    Y = const.tile([128, 4, 1024], F32, tag="Y")

    if variant == "dma":
        for m in range(4):
            nc.vector.tensor_copy(out=Y[:, m, :],
                                  in_=Xs[:, m, :].bitcast(F32))
        nc.sync.dma_start(out=io["y"], in_=Y[:])
        return

    # ---- Q/K projections: dst[inner, hw] = W^T.T @ src + b ----
    for dst, w, b, src in ((Q, WqT, bq, Xs), (K, WkT, bk, Cs)):
        for m in range(4):
            for n in range(2):
                ps = sp.tile([128, 1024], F32, tag="sp")
                for kc in range(4):
                    _mm(nc, ps[:, 0:512],
                        w[:, kc, m * 128:(m + 1) * 128],
                        src[:, kc, n * 512:(n + 1) * 512],
                        kc == 0, kc == 3)
                nc.scalar.add(dst[:, m, n * 512:(n + 1) * 512],
                              ps[:, 0:512], b[:, m:m + 1])

    # ---- V^T: Vt[hw, inner] = Ctx.T @ Wv^T + 1 (x) bv ----
    for jt in range(8):
        ps = sp.tile([128, 1024], F32, tag="sp")
        for kc in range(4):
            _mm(nc, ps[:, 0:512],
                Cs[:, kc, jt * 128:(jt + 1) * 128],
                WvT[:, kc, :],
                kc == 0, False)
        _mm(nc, ps[:, 0:512], ones, bv, False, True)
        nc.vector.tensor_copy(out=Vt[:, jt, :], in_=ps[:, 0:512])

    # ---- attention ----
    if variant == "proj":
        for m in range(4):
            nc.vector.tensor_copy(
                out=O[:, m, :],
                in_=Vt[:, 2 * m:2 * m + 2, :].rearrange("p a b -> p (a b)"))
    for h in range(N_HEADS if variant != "proj" else 0):
        m_h, p0 = h // 2, (h % 2) * 64
        Qh = Q[p0:p0 + 64, m_h, :]
        Kh = K[p0:p0 + 64, m_h, :]
        po = av.tile([64, 1024], F32, tag="av")
        for qt in range(8):
            ps = sp.tile([128, 1024], F32, tag="sp")
            qslice = Qh[:, qt * 128:(qt + 1) * 128]
            _mm(nc, ps[:, 0:512], qslice, Kh[:, 0:512], True, True)
            _mm(nc, ps[:, 512:1024], qslice, Kh[:, 512:1024], True, True)

            probs = ppool.tile([128, 1024], MM_DT, tag="probs")
            if variant == "noexp":
                nc.scalar.copy(probs[:], ps[:])
                vsc = Vt[:, qt, h * 64:(h + 1) * 64]
            else:
                sums = sm.tile([128, 1], F32, tag="sums")
                nc.scalar.activation(out=probs[:], in_=ps[:],
                                     func=mybir.ActivationFunctionType.Exp,
                                     accum_out=sums[:])
                rec = sm.tile([128, 1], F32, tag="rec")
                nc.vector.reciprocal(out=rec[:], in_=sums[:])
                vsc_t = vpool.tile([128, 64], MM_DT, tag="vsc")
                nc.vector.tensor_scalar_mul(vsc_t[:],
                                            Vt[:, qt, h * 64:(h + 1) * 64],
                                            rec[:])
                vsc = vsc_t[:]
            _mm(nc, po[:, 0:512], vsc, probs[:, 0:512], qt == 0, qt == 7)
            _mm(nc, po[:, 512:1024], vsc, probs[:, 512:1024],
                qt == 0, qt == 7)
        nc.vector.tensor_copy(out=O[p0:p0 + 64, m_h, :], in_=po[:])

    # ---- output projection: Y = Wo^T.T @ O + bo ----
    for m in range(4):
        for n in range(2):
            ps = sp.tile([128, 1024], F32, tag="sp")
            for kc in range(4):
                _mm(nc, ps[:, 0:512],
                    WoT[:, kc, m * 128:(m + 1) * 128],
                    O[:, kc, n * 512:(n + 1) * 512],
                    kc == 0, kc == 3)
            nc.scalar.add(Y[:, m, n * 512:(n + 1) * 512],
                          ps[:, 0:512], bo[:, m:m + 1])

    nc.sync.dma_start(out=io["y"], in_=Y[:])


def build_nc(repeat: int = 1, variant: str = "full"):
    nc = bacc.Bacc("TRN2", target_bir_lowering=False, debug=False)
    io = {
        "blob": nc.dram_tensor("blob", [128, BLOB_F], MM_DT,
                               kind="ExternalInput").ap(),
        "y": nc.dram_tensor("y", [128, 4, 1024], F32,
                            kind="ExternalOutput").ap(),
    }
    with tile.TileContext(nc) as tc:
        with ExitStack() as ctx:
            pools = make_pools(ctx, tc)
            for _ in range(repeat):
                _kernel_body(ctx, tc, io, pools, variant=variant)
    nc.compile()
    return nc


def _pack_cmajor(a: np.ndarray, nchunk: int) -> np.ndarray:
    """[nchunk*128, F] -> [128, nchunk*F] with row r = chunk*128 + p."""
    f = a.shape[1]
    return a.reshape(nchunk, 128, f).transpose(1, 0, 2).reshape(128, nchunk * f)


def make_in_maps(x, context, Wq, bq, Wk, bk, Wv, bv, Wo, bo):
    shared = np.zeros((128, BLOB_F), np.float32)
    shared[:, OFF_WQ:OFF_WQ + 2048] = _pack_cmajor(
        np.ascontiguousarray(Wq.T) / 8.0, 4)
    shared[:, OFF_WK:OFF_WK + 2048] = _pack_cmajor(
        np.ascontiguousarray(Wk.T), 4)
    shared[:, OFF_WV:OFF_WV + 2048] = _pack_cmajor(
        np.ascontiguousarray(Wv.T), 4)
    shared[:, OFF_WO:OFF_WO + 2048] = _pack_cmajor(
        np.ascontiguousarray(Wo.T), 4)
    shared[:, OFF_BQ:OFF_BQ + 4] = (bq / 8.0).reshape(4, 128).T
    shared[:, OFF_BK:OFF_BK + 4] = bk.reshape(4, 128).T
    shared[:, OFF_BO:OFF_BO + 4] = bo.reshape(4, 128).T
    shared[0, OFF_BV:OFF_BV + 512] = bv
    shared[0, OFF_ONES:OFF_ONES + 128] = 1.0

    in_maps = []
    for b in range(BS):
        blob = shared.copy()
        blob[:, OFF_X:OFF_X + 4096] = _pack_cmajor(x[b].reshape(C, HW), 4)
        blob[:, OFF_CTX:OFF_CTX + 4096] = _pack_cmajor(
            context[b].reshape(C, HW), 4)
        in_maps.append({"blob": blob})
    return in_maps


def kernel_with_results(inputs: dict, trace: bool = False, **run_kwargs):
    in_maps = make_in_maps(**{k: np.asarray(v, np.float32)
                              for k, v in inputs.items()})
    nc = build_nc()
    res = run_bass_kernel_spmd(nc, in_maps, core_ids=list(range(N_CORES)),
                               trace=trace, **run_kwargs)
    outs = []
    for r in res.results:
        y = r["y"]  # [128, 4, 1024]
        outs.append(y.transpose(1, 0, 2).reshape(C, H, W))
    return np.stack(outs).astype(np.float32), res


def kernel(**inputs) -> np.ndarray:
    out, _ = kernel_with_results(inputs)
    return out
